# revision 1
# baseline (speedup 1.0000x reference)
"""Trainium2 Bass kernel for nn_DivEncLayer (grouped tiny-MLP + ELU + LayerNorm + proj).

Math (per batch row b, slice q of Q=128, V=8, H=32):
    h   = elu(x[b,q,:] @ W1[q] + b1[q]);  hn = LN(h)*gamma[q]+beta[q]
    out[b,q] = hn @ W2[q] + b2[q]

Folded form (LN algebra -> 3 segmented reductions, all done by PE matmuls):
    g2c = gamma*W2 - mean(gamma*W2); c2 = sum(beta*W2)+b2
    s = sum_h(he), w = sum_h(he*g2c), t = sum_h(he^2)
    out = c2 + w * sqrt(H) / sqrt(t - s^2/H + H*eps)

Device layout: features on partitions, batch on free dim (host pre-transposes
x, so zero on-chip transposes). Per 512-batch supertile:
  - mm1: 32 block-diagonal [K=128, M=128, N=512] float32r matmuls (full rate)
  - ACT: emu = Exp(h+b1) (one pass)
  - DVE: het = relu(h+b1) + min(emu-1, 0)  == elu  (one fused custom op)
         he2 = het*het (bf16 tensor_tensor, 2x mode)
  - stats: 3 bf16 matmuls per tile, col-tiled (tile_position), zero-padded
    M=32 stationaries accumulating DENSE [128q, 512b] stats banks
  - finishing on dense banks: 2 custom DVE ops + Square/Ln/Exp on ACT
"""

import os
import sys

for _p in ("/opt/trn_rl_repo",):
    if _p not in sys.path:
        sys.path.insert(0, _p)

import numpy as np

B, Q, V, H = 32768, 128, 8, 32
N_CORES = 8
BC = B // N_CORES          # 4096 batch rows per core
SB = 512                   # supertile batch columns
NST = BC // SB             # 8 supertiles per core
LN_EPS = 1e-5

_CACHE = {}
_OPS_REGISTERED = False
_last_in_maps = None


def _q_of_r():
    # stats-bank row r = 32*t + 4*g + j  <->  q = 16*g + 4*t + j
    r = np.arange(128)
    t, g, j = r // 32, (r % 32) // 4, r % 4
    return (16 * g + 4 * t + j).astype(np.int64)


def _register_custom_ops():
    """Append our fused DVE ops to the dve_ops registry (self-pinned shas)."""
    global _OPS_REGISTERED
    import concourse.dve_ops as dve_ops
    from concourse.dve_ops import DveOp
    from concourse.dve_spec import C0, C1, Spec, Src0, Src1, Zero, lower, minn, relu
    from concourse.dve_uop import DveOpSpec

    if _OPS_REGISTERED:
        return {op.name: op for op in dve_ops.OPS}

    def _pin(name, spec, ref):
        spec = Spec(body=spec, reference=ref)
        shas = {}
        for ver in ("v3", "v4"):
            row = dve_ops._CUSTOM_DVE_ROW_BASE + len(dve_ops.OPS)
            tmp = DveOpSpec(name=name, opcode=row, uops=lower(spec, ver=ver),
                            rd1_en=True)
            shas[ver] = tmp.sha(ver)
        op = DveOp(name, spec, subdim=False, uops_sha=shas)
        dve_ops.OPS.append(op)
        dve_ops.CUSTOM_DVE_SPECS[name] = spec
        dve_ops._SUB_OPCODE_FOR_NAME[name] = dve_ops._CUSTOM_DVE_ROW_BASE + len(dve_ops.OPS) - 1
        return op

    # het = relu(h + b1) + min(emu - 1, 0)   (exact ELU given emu = exp(h+b1))
    _pin(
        "ELU_FUSE_ANT",
        relu(Src0 + C0) + minn(Src1 - C1, Zero),
        lambda in0, in1, s0, s1, imm2: np.maximum(in0.astype(np.float32) + s0, 0.0)
        + np.minimum(in1.astype(np.float32) - s1, 0.0),
    )
    # D = t - s^2*c0 + c1   (square fused on DVE; avoids ACT table switches)
    from concourse.dve_spec import sq
    _pin(
        "VAR_PREP_ANT",
        (Src0 - sq(Src1) * C0) + C1,
        lambda in0, in1, s0, s1, imm2: (in0.astype(np.float32) - in1.astype(np.float32) ** 2 * s0) + s1,
    )
    # out = rstd * w + c2
    _pin(
        "MUL_ADD_ANT",
        Src0 * Src1 + C0,
        lambda in0, in1, s0, s1, imm2: in0.astype(np.float32) * in1 + s0,
    )
    _OPS_REGISTERED = True
    return {op.name: op for op in dve_ops.OPS}

def _build_program(tile_dt_name: str, ablate: str = "", reps: int = 1):
    ab = set(ablate.split(",")) if ablate else set()
    import concourse.bacc as bacc
    import concourse.tile as tile
    from concourse import mybir

    ops = _register_custom_ops()

    f32 = mybir.dt.float32
    f32r = mybir.dt.float32r
    bf16 = mybir.dt.bfloat16
    tile_dt = getattr(mybir.dt, tile_dt_name)
    AF = mybir.ActivationFunctionType
    ALU = mybir.AluOpType

    nc = bacc.Bacc(
        "TRN2",
        target_bir_lowering=False,
        debug=False,
        enable_asserts=False,
        num_devices=N_CORES,
    )

    xT = nc.dram_tensor("xT", [Q * V, BC], f32r, kind="ExternalInput").ap()
    w1p = nc.dram_tensor("w1p", [128, 32 * 128], f32r, kind="ExternalInput").ap()
    sp = nc.dram_tensor("sp", [128, 32 * 128], f32r, kind="ExternalInput").ap()
    wp = nc.dram_tensor("wp", [128, 32 * 128], f32r, kind="ExternalInput").ap()
    b1p = nc.dram_tensor("b1p", [128, 32], f32, kind="ExternalInput").ap()
    c2p = nc.dram_tensor("c2p", [128, 1], f32, kind="ExternalInput").ap()
    outT = nc.dram_tensor("outT", [128, BC], f32, kind="ExternalOutput").ap()

    with tile.TileContext(nc) as tc:
        with (
            tc.tile_pool(name="wts", bufs=1) as wts,
            tc.tile_pool(name="xt", bufs=12) as xtp,
            tc.tile_pool(name="elu", bufs=4) as elu,
            tc.tile_pool(name="fin", bufs=2) as fin,
            tc.tile_pool(name="hep", bufs=2, space="PSUM") as hep,
            tc.tile_pool(name="stp", bufs=2, space="PSUM") as stp,
        ):
            w1s = wts.tile([128, 32 * 128], f32r)
            nc.sync.dma_start(out=w1s, in_=w1p)
            sps = wts.tile([128, 32 * 128], f32r)
            nc.sync.dma_start(out=sps, in_=sp)
            wps = wts.tile([128, 32 * 128], f32r)
            nc.sync.dma_start(out=wps, in_=wp)
            b1s = wts.tile([128, 32], f32)
            nc.sync.dma_start(out=b1s, in_=b1p)
            c2s = wts.tile([128, 1], f32)
            nc.sync.dma_start(out=c2s, in_=c2p)
            zero_c = wts.tile([128, 1], f32)
            nc.vector.memset(zero_c, 0.0)
            lnh_c = wts.tile([128, 1], f32)
            nc.vector.memset(lnh_c, float(0.5 * np.log(H)))

            import contextlib

            loop_cm = tc.For_i(0, reps, 1) if reps > 1 else contextlib.nullcontext()
            with loop_cm:
              for st in range(NST):
                xts = []
                for g in range(8):
                    xt_t = xtp.tile([128, SB], f32r, tag="xt")
                    nc.sync.dma_start(
                        out=xt_t, in_=xT[128 * g : 128 * g + 128, SB * st : SB * st + SB]
                    )
                    xts.append(xt_t)

                bankS = stp.tile([128, SB], f32, tag="bankS")
                bankW = stp.tile([128, SB], f32, tag="bankW")
                bankT = stp.tile([128, SB], f32, tag="bankT")

                for i in range(32):
                    g, t = i // 4, i % 4
                    he = hep.tile([128, SB], f32, tag="he")
                    nc.tensor.matmul(
                        he,
                        lhsT=w1s[:, 128 * i : 128 * i + 128],
                        rhs=xts[g],
                        start=True,
                        stop=True,
                    )
                    bias = b1s[:, i : i + 1]
                    emu = elu.tile([128, SB], f32, tag="emu")
                    nc.scalar.activation(emu, he, AF.Exp, bias=bias, scale=1.0)
                    het = elu.tile([128, SB], f32r, tag="het")
                    nc.vector._custom_dve(
                        ops["ELU_FUSE_ANT"], out=het, in0=he, in1=emu,
                        s0=bias, s1=1.0,
                    )
                    he2 = elu.tile([128, SB], f32r, tag="he2")
                    if "acthe2" in ab and i % 2 == 1:
                        nc.scalar.activation(he2, het, AF.Square, bias=zero_c[:, 0:1])
                    else:
                        nc.vector.tensor_mul(he2, het, het)

                    su = sps[:, 128 * i : 128 * i + 128]
                    wu = wps[:, 128 * i : 128 * i + 128]
                    first = i == 0
                    last = i == 31
                    if "stats" not in ab:
                        nc.tensor.matmul(bankS, lhsT=su, rhs=het, start=first, stop=last)
                        nc.tensor.matmul(bankW, lhsT=wu, rhs=het, start=first, stop=last)
                        nc.tensor.matmul(bankT, lhsT=su, rhs=he2, start=first, stop=last)
                    elif i == 0:
                        nc.tensor.matmul(bankS, lhsT=su, rhs=het, start=True, stop=True)
                        nc.tensor.matmul(bankW, lhsT=wu, rhs=het, start=True, stop=True)
                        nc.tensor.matmul(bankT, lhsT=su, rhs=he2, start=True, stop=True)

                # finishing: out = c2 + w * exp(0.5*ln(32) - 0.5*ln(D)),
                # D = t - s^2/32 + 32*eps
                z = fin.tile([128, SB], f32, tag="z")
                nc.vector.tensor_copy(z, bankS)
                D = fin.tile([128, SB], f32, tag="D")
                nc.vector._custom_dve(
                    ops["VAR_PREP_ANT"], out=D, in0=bankT, in1=z,
                    s0=1.0 / H, s1=float(H * LN_EPS),
                )
                L = fin.tile([128, SB], f32, tag="L")
                nc.scalar.activation(L, D, AF.Ln, bias=zero_c[:, 0:1])
                rstd = fin.tile([128, SB], f32, tag="rstd")
                nc.scalar.activation(rstd, L, AF.Exp, bias=lnh_c[:, 0:1], scale=-0.5)
                of = fin.tile([128, SB], f32, tag="of")
                nc.vector._custom_dve(
                    ops["MUL_ADD_ANT"], out=of, in0=rstd, in1=bankW,
                    s0=c2s[:, 0:1], s1=0.0,
                )
                nc.sync.dma_start(out=outT[:, SB * st : SB * st + SB], in_=of)

    nc.compile()
    return nc


def _host_pack(W1, b1, gamma, beta, W2, b2):
    import ml_dtypes

    g2 = (gamma * W2[:, :, 0]).astype(np.float64)
    g2c = (g2 - g2.sum(-1, keepdims=True) / H).astype(np.float32)
    c2 = ((beta * W2[:, :, 0]).sum(-1) + b2[:, 0]).astype(np.float32)

    w1p = np.zeros((128, 32 * 128), np.float32)
    sp = np.zeros((128, 32 * 128), np.float32)
    wp = np.zeros((128, 32 * 128), np.float32)
    b1p = np.zeros((128, 32), np.float32)
    for g in range(8):
        for t in range(4):
            i = 4 * g + t
            for j in range(4):
                q = 16 * g + 4 * t + j
                w1p[
                    32 * t + 8 * j : 32 * t + 8 * j + 8,
                    128 * i + 32 * j : 128 * i + 32 * j + 32,
                ] = W1[q]
                # bank row r = 32*t + 4*g + j; lhsT col m writes bank row m
                sp[32 * j : 32 * j + 32, 128 * i + 32 * t + 4 * g + j] = 1.0
                wp[32 * j : 32 * j + 32, 128 * i + 32 * t + 4 * g + j] = g2c[q]
                b1p[32 * j : 32 * j + 32, i] = b1[q]
    c2p = c2[_q_of_r()].reshape(128, 1).astype(np.float32)
    return (w1p, sp, wp, b1p, c2p)


def kernel(x, W1, b1, gamma, beta, W2, b2):
    from concourse import bass_utils

    tile_dt_name = os.environ.get("KERNEL_TILE_DT", "bfloat16")
    key = tile_dt_name
    if key not in _CACHE:
        _CACHE[key] = _build_program(tile_dt_name)
    nc = _CACHE[key]

    x = np.asarray(x, np.float32)
    w1p, sp, wp, b1p, c2p = _host_pack(
        np.asarray(W1, np.float32),
        np.asarray(b1, np.float32),
        np.asarray(gamma, np.float32),
        np.asarray(beta, np.float32),
        np.asarray(W2, np.float32),
        np.asarray(b2, np.float32),
    )

    in_maps = []
    for c in range(N_CORES):
        xc = x[BC * c : BC * (c + 1), :]          # [4096, 1024]
        in_maps.append(
            {
                "xT": np.ascontiguousarray(xc.T),  # [1024, 4096]
                "w1p": w1p,
                "sp": sp,
                "wp": wp,
                "b1p": b1p,
                "c2p": c2p,
            }
        )

    global _last_in_maps
    _last_in_maps = in_maps

    res = bass_utils.run_bass_kernel_spmd(
        nc, in_maps, core_ids=list(range(N_CORES))
    )

    qr = _q_of_r()
    out = np.empty((B, Q), np.float32)
    for c in range(N_CORES):
        blk = np.empty((BC, Q), np.float32)
        blk[:, qr] = res.results[c]["outT"].T
        out[BC * c : BC * (c + 1), :] = blk
    return out



# revision 28
# speedup vs baseline: 1342.3720x; 1342.3720x over previous
"""Trainium2 Bass kernel for nn_DivEncLayer (grouped tiny-MLP + ELU + LayerNorm + proj).

Math (per batch row b, slice q of Q=128, V=8, H=32):
    h   = elu(x[b,q,:] @ W1[q] + b1[q]);  hn = LN(h)*gamma[q]+beta[q]
    out[b,q] = hn @ W2[q] + b2[q]

Folded form (LN algebra -> 3 segmented reductions done by PE matmuls):
    g2c = gamma*W2 - mean(gamma*W2); c2 = sum(beta*W2)+b2
    y = elu(u)+1 (shift is free: sum_h g2c = 0, variance shift-invariant)
    s = sum_h y, w = sum_h g2c*y, t = sum_h (y-1)^2   [centered square ->
        no cancellation blowup, so bf16 stats inputs are safe]
    D = t - (s-H)^2/H + H*eps;  out = c2 + w * sqrt(H/D)

Device layout: features on partitions, batch on free dim. x is host-repacked
into 11 slices of 12 q (last 8 q) with a ones-row at slice row 96, so ONE
K=97 matmul per i-tile computes u + (b1+8) with no separate bias matmul.
i-tile i covers q = 4i+j, j=0..3.

Per 512-batch supertile:
  - mm1: 32 [K=97, M=128, N=512] f32r matmuls -> PSUM (pair tiles [128,1024])
  - ELU: ONE 8-stage custom DVE op per pair (no ACT exp): with m = relu(u+8),
        y = max(m-7, min(((m/8)^2^2)^2, 1))     [~(1+u/8)^8 ~ e^u for u<=0]
    reads PSUM f32, writes bf16
  - squares: ACT Square(y + (-1)) -> bf16 per pair
  - stats: 3 accumulating [K=128, M=128, N=512] bf16 matmuls per i-tile into
    banks S/W/T, bank row r = 32*(i%4) + 4*(i//4) + j
  - finishing per supertile: ACT Copy, custom VAR_PREP2, ACT Ln, ACT Exp,
    custom MUL_ADD  (all ACT funcs live in natural_log_exp_and_others ->
    exactly one table load)
"""

import os
import sys

for _p in ("/opt/trn_rl_repo",):
    if _p not in sys.path:
        sys.path.insert(0, _p)

import numpy as np

B, Q, V, H = 32768, 128, 8, 32
N_CORES = 8
BC = B // N_CORES          # 4096 batch rows per core
SB = 512                   # supertile batch columns
NST = BC // SB             # 8 supertiles per core
LN_EPS = 1e-5
EA = 16.0                  # ELU poly shift/power
SIG = float(16.0 ** (-16.0 / 15.0))   # global y-scale folded into W1 (power-16 poly)
C0_ELU = float((16.0 * SIG) ** 8)     # half-power clamp level
C1_ELU = float(15.0 * SIG)            # d-branch offset

NSLICE = 11                # x slices: 10 x 12q + 1 x 8q
SROWS = 97                 # rows per slice: 12*8 x-rows + ones row at 96
XROWS = NSLICE * SROWS     # 1067

_CACHE = {}
_OPS_REGISTERED = False
_last_in_maps = None


def _q_of_r():
    # stats-bank row r = 32*t + 4*g + j  <->  i = 4*g + t, q = 4*i + j = 16g+4t+j
    r = np.arange(128)
    t, g, j = r // 32, (r % 32) // 4, r % 4
    return (16 * g + 4 * t + j).astype(np.int64)


def _slice_of_i(i):
    """(slice index, tile-within-slice) for i-tile i (covers q = 4i..4i+3)."""
    if i < 30:
        return i // 3, i % 3
    return 10, i - 30


def _act_single_table(tables):
    """Zero out every act set except natural_log_exp_and_others (preserving
    indices) so the table-load fixpoint resolves all our ACT functions
    (Square/Copy/Ln/Exp) to one set -> a single load at kernel start."""
    keep = "natural_log_exp_and_others"
    return {name: (fns if name == keep else set()) for name, fns in tables.items()}


def _register_custom_ops():
    global _OPS_REGISTERED
    import concourse.dve_ops as dve_ops
    from concourse.dve_ops import DveOp
    from concourse.dve_spec import C0, C1, C2, One, Spec, Src0, Src1, lower, minn, maxx, relu, sq
    from concourse.dve_uop import DveOpSpec

    if _OPS_REGISTERED:
        return {op.name: op for op in dve_ops.OPS}

    def _pin(name, spec, ref):
        spec = Spec(body=spec, reference=ref)
        shas = {}
        for ver in ("v3", "v4"):
            row = dve_ops._CUSTOM_DVE_ROW_BASE + len(dve_ops.OPS)
            tmp = DveOpSpec(name=name, opcode=row, uops=lower(spec, ver=ver),
                            rd1_en=True)
            shas[ver] = tmp.sha(ver)
        op = DveOp(name, spec, subdim=False, uops_sha=shas)
        dve_ops.OPS.append(op)
        dve_ops.CUSTOM_DVE_SPECS[name] = spec
        dve_ops._SUB_OPCODE_FOR_NAME[name] = dve_ops._CUSTOM_DVE_ROW_BASE + len(dve_ops.OPS) - 1
        return op

    # y' = SIG*(elu(u)+1) approx from Src0 = SIG*(u + 16)   [SIG = 16^(-16/15)]
    #   m = relu(Src0); y' = max(Src0 - C1, sq(min(sq(sq(sq(m))), C0)))
    #   C0 = (16*SIG)^8 (half-power clamp), C1 = 15*SIG
    def _poly_ref(in0, in1, s0, s1, imm2):
        x = in0.astype(np.float32)
        m = np.maximum(x, 0.0)
        m8 = ((m * m) ** 2) ** 2
        pc = np.minimum(m8, s0)
        return np.maximum(x - s1, pc * pc)

    _pin(
        "POLY_ELU1_ANT",
        maxx(Src0 - C1, sq(minn(sq(sq(sq(relu(Src0)))), C0))),
        _poly_ref,
    )
    # D = (t - sq(s - C2)*C0) + C1   (VAR_PREP on centered stats)
    _pin(
        "VAR_PREP2_ANT",
        (Src0 - sq(Src1 - C2) * C0) + C1,
        lambda in0, in1, s0, s1, imm2: (
            in0.astype(np.float32) - (in1.astype(np.float32) - imm2) ** 2 * s0
        ) + s1,
    )
    # out = rstd * w + c2
    _pin(
        "MUL_ADD_ANT",
        Src0 * Src1 + C0,
        lambda in0, in1, s0, s1, imm2: in0.astype(np.float32) * in1 + s0,
    )
    _OPS_REGISTERED = True
    return {op.name: op for op in dve_ops.OPS}


def _build_program(tile_dt_name: str, ablate: str = "", reps: int = 1):
    ab = set(ablate.split(",")) if ablate else set()
    import concourse.bacc as bacc
    import concourse.tile as tile
    from concourse import mybir

    ops = _register_custom_ops()

    f32 = mybir.dt.float32
    f32r = mybir.dt.float32r
    bf16 = mybir.dt.bfloat16
    AF = mybir.ActivationFunctionType

    nc = bacc.Bacc(
        "TRN2",
        target_bir_lowering=False,
        debug=False,
        enable_asserts=False,
        num_devices=N_CORES,
    )

    f16 = mybir.dt.float16

    xT = nc.dram_tensor("xT", [XROWS, BC], f32r, kind="ExternalInput").ap()
    w1p = nc.dram_tensor("w1p", [SROWS, 32 * 128], f32r, kind="ExternalInput").ap()
    swc = nc.dram_tensor("swc", [128, 32 * 128], f16, kind="ExternalInput").ap()
    tsc = nc.dram_tensor("tsc", [128, 32 * 128], f16, kind="ExternalInput").ap()
    c2p = nc.dram_tensor("c2p", [128, 1], f32, kind="ExternalInput").ap()
    outT = nc.dram_tensor("outT", [128, BC], f32, kind="ExternalOutput").ap()

    with tile.TileContext(nc) as tc:
        with (
            tc.tile_pool(name="wts", bufs=1) as wts,
            tc.tile_pool(name="xt", bufs=16) as xtp,
            tc.tile_pool(name="het", bufs=4) as hetp,
            tc.tile_pool(name="sqp", bufs=4) as sqpp,
            tc.tile_pool(name="fin", bufs=2) as fin,
            tc.tile_pool(name="hep", bufs=2, space="PSUM") as hep,
            tc.tile_pool(name="stp", bufs=1, space="PSUM") as stp,
        ):
            w1s = wts.tile([SROWS, 32 * 128], f32r)
            nc.sync.dma_start(out=w1s, in_=w1p)
            sws = wts.tile([128, 32 * 128], f16)
            nc.sync.dma_start(out=sws, in_=swc)
            tss = wts.tile([128, 32 * 128], f16)
            nc.sync.dma_start(out=tss, in_=tsc)
            c2s = wts.tile([128, 1], f32)
            nc.sync.dma_start(out=c2s, in_=c2p)
            zero_c = wts.tile([128, 1], f32)
            nc.vector.memset(zero_c, 0.0)
            negone_c = wts.tile([128, 1], f32)
            nc.vector.memset(negone_c, -1.0)
            lnh_c = wts.tile([128, 1], f32)
            nc.vector.memset(lnh_c, float(0.5 * np.log(H) - np.log(SIG)))

            def emit_fin(bankSW1, bankSW2, bankT, st):
                # gather s rows (low half of each 32-window) and w rows (high
                # half) from SW1 (g<4) / SW2 (g>=4) into lane-aligned SBUF
                # tiles via partition-shifting strip DMAs
                cS1 = fin.tile([128, SB], f32, tag="cS1")
                nc.scalar.activation(cS1, bankSW1, AF.Copy, bias=0.0)
                cS2 = fin.tile([128, SB], f32, tag="cS2")
                nc.scalar.activation(cS2, bankSW2, AF.Copy, bias=0.0)
                zt = fin.tile([128, SB], f32, tag="zt")
                wt = fin.tile([128, SB], f32, tag="wt")
                for t in range(4):
                    lo, hi = 32 * t, 32 * t + 16
                    nc.sync.dma_start(out=zt[lo:hi, :], in_=cS1[lo:hi, :])
                    nc.sync.dma_start(out=zt[hi : hi + 16, :], in_=cS2[lo:hi, :])
                    nc.sync.dma_start(out=wt[lo:hi, :], in_=cS1[hi : hi + 16, :])
                    nc.sync.dma_start(out=wt[hi : hi + 16, :], in_=cS2[hi : hi + 16, :])
                # D = t2 - ((s-H*SIG)/SIG)^2/H + H*eps; out = c2 + (w/SIG)*sqrt(H/D)
                Dt = fin.tile([128, SB], f32, tag="Dt")
                nc.vector._custom_dve(
                    ops["VAR_PREP2_ANT"], out=Dt, in0=bankT, in1=zt,
                    s0=float(1.0 / (H * SIG * SIG)), s1=float(H * LN_EPS),
                    imm2=float(H * SIG),
                )
                Lt = fin.tile([128, SB], f32, tag="Lt")
                nc.scalar.activation(Lt, Dt, AF.Ln, bias=zero_c[:, 0:1])
                rstd = fin.tile([128, SB], f32, tag="rstd")
                nc.scalar.activation(rstd, Lt, AF.Exp, bias=lnh_c[:, 0:1], scale=-0.5)
                of = fin.tile([128, SB], f32, tag="of")
                nc.vector._custom_dve(
                    ops["MUL_ADD_ANT"], out=of, in0=rstd, in1=wt,
                    s0=c2s[:, 0:1], s1=0.0,
                )
                nc.sync.dma_start(out=outT[:, SB * st : SB * st + SB], in_=of)

            FIN_DELAY = 4   # i-tiles of the next supertile emitted before fin
            for _rep in range(reps):
              pending = None
              for st in range(NST):
                xts = []
                for s in range(NSLICE):
                    xt_t = xtp.tile([SROWS, SB], f32r, tag="xt")
                    nc.sync.dma_start(
                        out=xt_t,
                        in_=xT[SROWS * s : SROWS * s + SROWS, SB * st : SB * st + SB],
                    )
                    xts.append(xt_t)

                bankSW1 = stp.tile([128, SB], f32, tag="bankSW1")
                bankSW2 = stp.tile([128, SB], f32, tag="bankSW2")
                bankT = stp.tile([128, SB], f32, tag="bankT")

                def emit_stats(i, hsl, qsl):
                    bsw = bankSW1 if i < 16 else bankSW2
                    nc.tensor.matmul(
                        bsw, lhsT=sws[:, 128 * i : 128 * i + 128], rhs=hsl,
                        start=(i % 16 == 0), stop=(i % 16 == 15),
                    )
                    nc.tensor.matmul(
                        bankT, lhsT=tss[:, 128 * i : 128 * i + 128], rhs=qsl,
                        start=(i == 0), stop=(i == 31),
                    )

                pend_stats = []
                for pr in range(16):
                    he = hep.tile([128, 2 * SB], f32, tag="he")
                    for z in range(2):
                        i = 2 * pr + z
                        s, _tl = _slice_of_i(i)
                        nc.tensor.matmul(
                            he[:, SB * z : SB * z + SB],
                            lhsT=w1s[:, 128 * i : 128 * i + 128],
                            rhs=xts[s],
                            start=True,
                            stop=True,
                        )
                    het1 = hetp.tile([128, 2 * SB], f16, tag="het1")
                    nc.vector._custom_dve(
                        ops["POLY_ELU1_ANT"], out=het1, in0=he,
                        s0=C0_ELU, s1=C1_ELU,
                    )
                    # (y'/SIG - 1)^2 = (y-1)^2 in fp16 via ACT free affine
                    sq1 = sqpp.tile([128, 2 * SB], f16, tag="sq1")
                    nc.scalar.activation(
                        sq1, het1, AF.Square, bias=negone_c[:, 0:1],
                        scale=float(1.0 / SIG),
                    )

                    for z in range(2):
                        i = 2 * pr + z
                        hsl = het1[:, SB * z : SB * z + SB]
                        qsl = sq1[:, SB * z : SB * z + SB]
                        if pending is not None and pr < FIN_DELAY:
                            pend_stats.append(
                                lambda i=i, h=hsl, q=qsl: emit_stats(i, h, q)
                            )
                        else:
                            if pending is not None:
                                emit_fin(*pending)
                                pending = None
                                for f in pend_stats:
                                    f()
                                pend_stats = []
                            emit_stats(i, hsl, qsl)

                pending = (bankSW1, bankSW2, bankT, st)
              emit_fin(*pending)

    # Compile with the act-table chooser pinned to one set so walrus emits a
    # single InstLoadActFuncSet instead of ping-ponging between sets.
    import concourse.hw_specs as hw_specs

    orig = hw_specs.get_activation_tables
    patched = lambda arch: _act_single_table(orig(arch))
    hw_specs_get = hw_specs.get_activation_tables
    try:
        hw_specs.get_activation_tables = patched
        bacc.get_activation_tables = patched
        nc.compile()
    finally:
        hw_specs.get_activation_tables = hw_specs_get
        bacc.get_activation_tables = hw_specs_get
    return nc


def _host_pack(W1, b1, gamma, beta, W2, b2):
    f16dt = np.float16
    g2 = (gamma * W2[:, :, 0]).astype(np.float64)
    g2c = (g2 - g2.sum(-1, keepdims=True) / H).astype(np.float32)
    c2 = ((beta * W2[:, :, 0]).sum(-1) + b2[:, 0]).astype(np.float32)

    sig = np.float32(SIG)
    w1p = np.zeros((SROWS, 32 * 128), np.float32)
    swc = np.zeros((128, 32 * 128), np.float32)
    tsc = np.zeros((128, 32 * 128), np.float32)
    for i in range(32):
        s, tl = _slice_of_i(i)
        g, t = i // 4, i % 4
        gl = g % 4
        for j in range(4):
            q = 4 * i + j
            ql = 4 * tl + j                      # q-local index within slice
            w1p[8 * ql : 8 * ql + 8, 128 * i + 32 * j : 128 * i + 32 * j + 32] = (
                W1[q] * sig
            )
            w1p[96, 128 * i + 32 * j : 128 * i + 32 * j + 32] = sig * (b1[q] + EA)
            r = 32 * t + 4 * g + j               # fin lane for q
            rs = 32 * t + 4 * gl + j             # s row within SW bank half
            swc[32 * j : 32 * j + 32, 128 * i + rs] = 1.0          # s selector
            swc[32 * j : 32 * j + 32, 128 * i + rs + 16] = g2c[q]  # w selector
            tsc[32 * j : 32 * j + 32, 128 * i + r] = 1.0           # t selector
    c2p = c2[_q_of_r()].reshape(128, 1).astype(np.float32)
    return (w1p, swc.astype(f16dt), tsc.astype(f16dt), c2p)


def _pack_x(xc):
    """[BC, 1024] core slice -> [XROWS, BC] with ones rows at slice row 96."""
    xp = np.zeros((XROWS, xc.shape[0]), np.float32)
    xcT = xc.T  # [1024, BC]; original row = 8*q + v
    for s in range(NSLICE):
        nq = 12 if s < 10 else 8
        q0 = 12 * s
        src = xcT[8 * q0 : 8 * (q0 + nq), :]
        xp[SROWS * s : SROWS * s + 8 * nq, :] = src
        xp[SROWS * s + 96, :] = 1.0
    return xp


def kernel(x, W1, b1, gamma, beta, W2, b2):
    from concourse import bass_utils

    tile_dt_name = os.environ.get("KERNEL_TILE_DT", "bfloat16")
    key = tile_dt_name
    if key not in _CACHE:
        _CACHE[key] = _build_program(tile_dt_name)
    nc = _CACHE[key]

    x = np.asarray(x, np.float32)
    w1p, swc, tsc, c2p = _host_pack(
        np.asarray(W1, np.float32),
        np.asarray(b1, np.float32),
        np.asarray(gamma, np.float32),
        np.asarray(beta, np.float32),
        np.asarray(W2, np.float32),
        np.asarray(b2, np.float32),
    )

    in_maps = []
    for c in range(N_CORES):
        xc = x[BC * c : BC * (c + 1), :]          # [4096, 1024]
        in_maps.append(
            {
                "xT": _pack_x(xc),
                "w1p": w1p,
                "swc": swc,
                "tsc": tsc,
                "c2p": c2p,
            }
        )

    global _last_in_maps
    _last_in_maps = in_maps

    res = bass_utils.run_bass_kernel_spmd(
        nc, in_maps, core_ids=list(range(N_CORES))
    )

    qr = _q_of_r()
    out = np.empty((B, Q), np.float32)
    for c in range(N_CORES):
        blk = np.empty((BC, Q), np.float32)
        blk[:, qr] = res.results[c]["outT"].T
        out[BC * c : BC * (c + 1), :] = blk
    return out


# revision 30
# speedup vs baseline: 1455.3749x; 1.0842x over previous
"""Trainium2 Bass kernel for nn_DivEncLayer (grouped tiny-MLP + ELU + LayerNorm + proj).

Math (per batch row b, slice q of Q=128, V=8, H=32):
    h   = elu(x[b,q,:] @ W1[q] + b1[q]);  hn = LN(h)*gamma[q]+beta[q]
    out[b,q] = hn @ W2[q] + b2[q]

Folded form (LN algebra -> 3 segmented reductions done by PE matmuls):
    g2c = gamma*W2 - mean(gamma*W2); c2 = sum(beta*W2)+b2
    y = elu(u)+1 (shift is free: sum_h g2c = 0, variance shift-invariant)
    s = sum_h y, w = sum_h g2c*y, t = sum_h (y-1)^2   [centered square ->
        no cancellation blowup, so bf16 stats inputs are safe]
    D = t - (s-H)^2/H + H*eps;  out = c2 + w * sqrt(H/D)

Device layout: features on partitions, batch on free dim. x is host-repacked
into 11 slices of 12 q (last 8 q) with a ones-row at slice row 96, so ONE
K=97 matmul per i-tile computes u + (b1+8) with no separate bias matmul.
i-tile i covers q = 4i+j, j=0..3.

Per 512-batch supertile:
  - mm1: 32 [K=97, M=128, N=512] f32r matmuls -> PSUM (pair tiles [128,1024])
  - ELU: ONE 8-stage custom DVE op per pair (no ACT exp): with m = relu(u+8),
        y = max(m-7, min(((m/8)^2^2)^2, 1))     [~(1+u/8)^8 ~ e^u for u<=0]
    reads PSUM f32, writes bf16
  - squares: ACT Square(y + (-1)) -> bf16 per pair
  - stats: 3 accumulating [K=128, M=128, N=512] bf16 matmuls per i-tile into
    banks S/W/T, bank row r = 32*(i%4) + 4*(i//4) + j
  - finishing per supertile: ACT Copy, custom VAR_PREP2, ACT Ln, ACT Exp,
    custom MUL_ADD  (all ACT funcs live in natural_log_exp_and_others ->
    exactly one table load)
"""

import os
import sys

for _p in ("/opt/trn_rl_repo",):
    if _p not in sys.path:
        sys.path.insert(0, _p)

import numpy as np

B, Q, V, H = 32768, 128, 8, 32
N_CORES = 8
BC = B // N_CORES          # 4096 batch rows per core
SB = 512                   # supertile batch columns
NST = BC // SB             # 8 supertiles per core
LN_EPS = 1e-5
EA = 16.0                  # ELU poly shift/power
SIG = float(16.0 ** (-16.0 / 15.0))   # global y-scale folded into W1 (power-16 poly)
C0_ELU = float((16.0 * SIG) ** 8)     # half-power clamp level
C1_ELU = float(15.0 * SIG)            # d-branch offset

NSLICE = 11                # x slices: 10 x 12q + 1 x 8q
SROWS = 97                 # rows per slice: 12*8 x-rows + ones row at 96
XROWS = NSLICE * SROWS     # 1067

_CACHE = {}
_OPS_REGISTERED = False
_last_in_maps = None


def _q_of_r():
    # fin lane r = 64*h + 16*t + 4*gl + j with g = gl + 4*h;  q = 16*g + 4*t + j
    r = np.arange(128)
    h, t, gl, j = r // 64, (r % 64) // 16, (r % 16) // 4, r % 4
    return (16 * (gl + 4 * h) + 4 * t + j).astype(np.int64)


def _slice_of_i(i):
    """(slice index, tile-within-slice) for i-tile i (covers q = 4i..4i+3)."""
    if i < 30:
        return i // 3, i % 3
    return 10, i - 30


def _act_single_table(tables):
    """Zero out every act set except natural_log_exp_and_others (preserving
    indices) so the table-load fixpoint resolves all our ACT functions
    (Square/Copy/Ln/Exp) to one set -> a single load at kernel start."""
    keep = "natural_log_exp_and_others"
    return {name: (fns if name == keep else set()) for name, fns in tables.items()}


def _register_custom_ops():
    global _OPS_REGISTERED
    import concourse.dve_ops as dve_ops
    from concourse.dve_ops import DveOp
    from concourse.dve_spec import C0, C1, C2, One, Spec, Src0, Src1, lower, minn, maxx, relu, sq
    from concourse.dve_uop import DveOpSpec

    if _OPS_REGISTERED:
        return {op.name: op for op in dve_ops.OPS}

    def _pin(name, spec, ref):
        spec = Spec(body=spec, reference=ref)
        shas = {}
        for ver in ("v3", "v4"):
            row = dve_ops._CUSTOM_DVE_ROW_BASE + len(dve_ops.OPS)
            tmp = DveOpSpec(name=name, opcode=row, uops=lower(spec, ver=ver),
                            rd1_en=True)
            shas[ver] = tmp.sha(ver)
        op = DveOp(name, spec, subdim=False, uops_sha=shas)
        dve_ops.OPS.append(op)
        dve_ops.CUSTOM_DVE_SPECS[name] = spec
        dve_ops._SUB_OPCODE_FOR_NAME[name] = dve_ops._CUSTOM_DVE_ROW_BASE + len(dve_ops.OPS) - 1
        return op

    # y' = SIG*(elu(u)+1) approx from Src0 = SIG*(u + 16)   [SIG = 16^(-16/15)]
    #   m = relu(Src0); y' = max(Src0 - C1, sq(min(sq(sq(sq(m))), C0)))
    #   C0 = (16*SIG)^8 (half-power clamp), C1 = 15*SIG
    def _poly_ref(in0, in1, s0, s1, imm2):
        x = in0.astype(np.float32)
        m = np.maximum(x, 0.0)
        m8 = ((m * m) ** 2) ** 2
        pc = np.minimum(m8, s0)
        return np.maximum(x - s1, pc * pc)

    _pin(
        "POLY_ELU1_ANT",
        maxx(Src0 - C1, sq(minn(sq(sq(sq(relu(Src0)))), C0))),
        _poly_ref,
    )
    # D = (t - sq(s - C2)*C0) + C1   (VAR_PREP on centered stats)
    _pin(
        "VAR_PREP2_ANT",
        (Src0 - sq(Src1 - C2) * C0) + C1,
        lambda in0, in1, s0, s1, imm2: (
            in0.astype(np.float32) - (in1.astype(np.float32) - imm2) ** 2 * s0
        ) + s1,
    )
    # out = rstd * w + c2
    _pin(
        "MUL_ADD_ANT",
        Src0 * Src1 + C0,
        lambda in0, in1, s0, s1, imm2: in0.astype(np.float32) * in1 + s0,
    )
    _OPS_REGISTERED = True
    return {op.name: op for op in dve_ops.OPS}


def _build_program(tile_dt_name: str, ablate: str = "", reps: int = 1):
    ab = set(ablate.split(",")) if ablate else set()
    import concourse.bacc as bacc
    import concourse.tile as tile
    from concourse import mybir

    ops = _register_custom_ops()

    f32 = mybir.dt.float32
    f32r = mybir.dt.float32r
    bf16 = mybir.dt.bfloat16
    AF = mybir.ActivationFunctionType

    nc = bacc.Bacc(
        "TRN2",
        target_bir_lowering=False,
        debug=False,
        enable_asserts=False,
        num_devices=N_CORES,
    )

    f16 = mybir.dt.float16

    xT = nc.dram_tensor("xT", [XROWS, BC], f32r, kind="ExternalInput").ap()
    w1p = nc.dram_tensor("w1p", [SROWS, 32 * 128], f32r, kind="ExternalInput").ap()
    swc = nc.dram_tensor("swc", [128, 32 * 128], f16, kind="ExternalInput").ap()
    tsc = nc.dram_tensor("tsc", [128, 32 * 128], f16, kind="ExternalInput").ap()
    c2p = nc.dram_tensor("c2p", [128, 1], f32, kind="ExternalInput").ap()
    outT = nc.dram_tensor("outT", [128, BC], f32, kind="ExternalOutput").ap()

    with tile.TileContext(nc) as tc:
        with (
            tc.tile_pool(name="wts", bufs=1) as wts,
            tc.tile_pool(name="xt", bufs=14) as xtp,
            tc.tile_pool(name="het", bufs=4) as hetp,
            tc.tile_pool(name="sqp", bufs=4) as sqpp,
            tc.tile_pool(name="fin", bufs=2) as fin,
            tc.tile_pool(name="hep", bufs=2, space="PSUM") as hep,
            tc.tile_pool(name="stp", bufs=1, space="PSUM") as stp,
        ):
            w1s = wts.tile([SROWS, 32 * 128], f32r)
            nc.sync.dma_start(out=w1s, in_=w1p)
            sws = wts.tile([128, 32 * 128], f16)
            nc.sync.dma_start(out=sws, in_=swc)
            tss = wts.tile([128, 32 * 128], f16)
            nc.sync.dma_start(out=tss, in_=tsc)
            c2s = wts.tile([128, 1], f32)
            nc.sync.dma_start(out=c2s, in_=c2p)
            zero_c = wts.tile([128, 1], f32)
            nc.vector.memset(zero_c, 0.0)
            negone_c = wts.tile([128, 1], f32)
            nc.vector.memset(negone_c, -1.0)
            lnh_c = wts.tile([128, 1], f32)
            nc.vector.memset(lnh_c, float(0.5 * np.log(H) - np.log(SIG)))

            def emit_fin(bankSW1, bankSW2, bankT, st):
                # gather s rows (low half of each 32-window) and w rows (high
                # half) from SW1 (g<4) / SW2 (g>=4) into lane-aligned SBUF
                # tiles via partition-shifting strip DMAs
                cS1 = fin.tile([128, SB], f32, tag="cS1")
                nc.scalar.activation(cS1, bankSW1, AF.Copy, bias=0.0)
                cS2 = fin.tile([128, SB], f32, tag="cS2")
                nc.scalar.activation(cS2, bankSW2, AF.Copy, bias=0.0)
                zt = fin.tile([128, SB], f32, tag="zt")
                wt = fin.tile([128, SB], f32, tag="wt")
                nc.sync.dma_start(out=zt[0:64, :], in_=cS1[0:64, :])
                nc.sync.dma_start(out=zt[64:128, :], in_=cS2[0:64, :])
                nc.sync.dma_start(out=wt[0:64, :], in_=cS1[64:128, :])
                nc.sync.dma_start(out=wt[64:128, :], in_=cS2[64:128, :])
                # D = t2 - ((s-H*SIG)/SIG)^2/H + H*eps; out = c2 + (w/SIG)*sqrt(H/D)
                Dt = fin.tile([128, SB], f32, tag="Dt")
                nc.vector._custom_dve(
                    ops["VAR_PREP2_ANT"], out=Dt, in0=bankT, in1=zt,
                    s0=float(1.0 / (H * SIG * SIG)), s1=float(H * LN_EPS),
                    imm2=float(H * SIG),
                )
                Lt = fin.tile([128, SB], f32, tag="Lt")
                nc.scalar.activation(Lt, Dt, AF.Ln, bias=zero_c[:, 0:1])
                rstd = fin.tile([128, SB], f32, tag="rstd")
                nc.scalar.activation(rstd, Lt, AF.Exp, bias=lnh_c[:, 0:1], scale=-0.5)
                of = fin.tile([128, SB], f32, tag="of")
                nc.vector._custom_dve(
                    ops["MUL_ADD_ANT"], out=of, in0=rstd, in1=wt,
                    s0=c2s[:, 0:1], s1=0.0,
                )
                nc.sync.dma_start(out=outT[:, SB * st : SB * st + SB], in_=of)

            FIN_DELAY = 4   # i-tiles of the next supertile emitted before fin
            for _rep in range(reps):
              pending = None
              xts2 = None
              for st in range(NST):
                if st % 2 == 0:
                    xts2 = []
                    for s in range(NSLICE):
                        xt_t = xtp.tile([SROWS, 2 * SB], f32r, tag="xt")
                        nc.sync.dma_start(
                            out=xt_t,
                            in_=xT[
                                SROWS * s : SROWS * s + SROWS,
                                SB * st : SB * st + 2 * SB,
                            ],
                        )
                        xts2.append(xt_t)
                xts = [xt[:, SB * (st % 2) : SB * (st % 2) + SB] for xt in xts2]

                bankSW1 = stp.tile([128, SB], f32, tag="bankSW1")
                bankSW2 = stp.tile([128, SB], f32, tag="bankSW2")
                bankT = stp.tile([128, SB], f32, tag="bankT")

                def emit_stats(i, hsl, qsl, part):
                    if part == 0:
                        bsw = bankSW1 if i < 16 else bankSW2
                        nc.tensor.matmul(
                            bsw, lhsT=sws[:, 128 * i : 128 * i + 128], rhs=hsl,
                            start=(i % 16 == 0), stop=(i % 16 == 15),
                        )
                    else:
                        nc.tensor.matmul(
                            bankT, lhsT=tss[:, 128 * i : 128 * i + 128], rhs=qsl,
                            start=(i == 0), stop=(i == 31),
                        )

                pend_stats = []
                for pr in range(16):
                    he = hep.tile([128, 2 * SB], f32, tag="he")
                    for z in range(2):
                        i = 2 * pr + z
                        s, _tl = _slice_of_i(i)
                        nc.tensor.matmul(
                            he[:, SB * z : SB * z + SB],
                            lhsT=w1s[:, 128 * i : 128 * i + 128],
                            rhs=xts[s],
                            start=True,
                            stop=True,
                        )
                    het1 = hetp.tile([128, 2 * SB], f16, tag="het1")
                    nc.vector._custom_dve(
                        ops["POLY_ELU1_ANT"], out=het1, in0=he,
                        s0=C0_ELU, s1=C1_ELU,
                    )
                    sq1 = sqpp.tile([128, 2 * SB], f16, tag="sq1")

                    for part in range(2):
                        if part == 1:
                            # (y'/SIG - 1)^2 = (y-1)^2 via ACT free affine;
                            # emitted after the SW stats so the square overlaps
                            # them on the PE
                            nc.scalar.activation(
                                sq1, het1, AF.Square, bias=negone_c[:, 0:1],
                                scale=float(1.0 / SIG),
                            )      # 0: SW stats, 1: T stats
                        for z in range(2):
                            i = 2 * pr + z
                            hsl = het1[:, SB * z : SB * z + SB]
                            qsl = sq1[:, SB * z : SB * z + SB]
                            if pending is not None and pr < FIN_DELAY:
                                pend_stats.append(
                                    lambda i=i, h=hsl, q=qsl, p=part: emit_stats(i, h, q, p)
                                )
                            else:
                                if pending is not None:
                                    emit_fin(*pending)
                                    pending = None
                                    for f in pend_stats:
                                        f()
                                    pend_stats = []
                                emit_stats(i, hsl, qsl, part)

                pending = (bankSW1, bankSW2, bankT, st)
              emit_fin(*pending)

    # Compile with the act-table chooser pinned to one set so walrus emits a
    # single InstLoadActFuncSet instead of ping-ponging between sets.
    import concourse.hw_specs as hw_specs

    orig = hw_specs.get_activation_tables
    patched = lambda arch: _act_single_table(orig(arch))
    hw_specs_get = hw_specs.get_activation_tables
    try:
        hw_specs.get_activation_tables = patched
        bacc.get_activation_tables = patched
        nc.compile()
    finally:
        hw_specs.get_activation_tables = hw_specs_get
        bacc.get_activation_tables = hw_specs_get
    return nc


def _host_pack(W1, b1, gamma, beta, W2, b2):
    f16dt = np.float16
    g2 = (gamma * W2[:, :, 0]).astype(np.float64)
    g2c = (g2 - g2.sum(-1, keepdims=True) / H).astype(np.float32)
    c2 = ((beta * W2[:, :, 0]).sum(-1) + b2[:, 0]).astype(np.float32)

    sig = np.float32(SIG)
    w1p = np.zeros((SROWS, 32 * 128), np.float32)
    swc = np.zeros((128, 32 * 128), np.float32)
    tsc = np.zeros((128, 32 * 128), np.float32)
    for i in range(32):
        s, tl = _slice_of_i(i)
        g, t = i // 4, i % 4
        gl = g % 4
        for j in range(4):
            q = 4 * i + j
            ql = 4 * tl + j                      # q-local index within slice
            w1p[8 * ql : 8 * ql + 8, 128 * i + 32 * j : 128 * i + 32 * j + 32] = (
                W1[q] * sig
            )
            w1p[96, 128 * i + 32 * j : 128 * i + 32 * j + 32] = sig * (b1[q] + EA)
            rs = 16 * t + 4 * gl + j             # s row (0..63) in SW bank
            r = 64 * (g // 4) + rs               # fin lane for q
            swc[32 * j : 32 * j + 32, 128 * i + rs] = 1.0          # s selector
            swc[32 * j : 32 * j + 32, 128 * i + 64 + rs] = g2c[q]  # w selector
            tsc[32 * j : 32 * j + 32, 128 * i + r] = 1.0           # t selector
    c2p = c2[_q_of_r()].reshape(128, 1).astype(np.float32)
    return (w1p, swc.astype(f16dt), tsc.astype(f16dt), c2p)


def _pack_x(xc):
    """[BC, 1024] core slice -> [XROWS, BC] with ones rows at slice row 96."""
    xp = np.zeros((XROWS, xc.shape[0]), np.float32)
    xcT = xc.T  # [1024, BC]; original row = 8*q + v
    for s in range(NSLICE):
        nq = 12 if s < 10 else 8
        q0 = 12 * s
        src = xcT[8 * q0 : 8 * (q0 + nq), :]
        xp[SROWS * s : SROWS * s + 8 * nq, :] = src
        xp[SROWS * s + 96, :] = 1.0
    return xp


def kernel(x, W1, b1, gamma, beta, W2, b2):
    from concourse import bass_utils

    tile_dt_name = os.environ.get("KERNEL_TILE_DT", "bfloat16")
    key = tile_dt_name
    if key not in _CACHE:
        _CACHE[key] = _build_program(tile_dt_name)
    nc = _CACHE[key]

    x = np.asarray(x, np.float32)
    w1p, swc, tsc, c2p = _host_pack(
        np.asarray(W1, np.float32),
        np.asarray(b1, np.float32),
        np.asarray(gamma, np.float32),
        np.asarray(beta, np.float32),
        np.asarray(W2, np.float32),
        np.asarray(b2, np.float32),
    )

    in_maps = []
    for c in range(N_CORES):
        xc = x[BC * c : BC * (c + 1), :]          # [4096, 1024]
        in_maps.append(
            {
                "xT": _pack_x(xc),
                "w1p": w1p,
                "swc": swc,
                "tsc": tsc,
                "c2p": c2p,
            }
        )

    global _last_in_maps
    _last_in_maps = in_maps

    res = bass_utils.run_bass_kernel_spmd(
        nc, in_maps, core_ids=list(range(N_CORES))
    )

    qr = _q_of_r()
    out = np.empty((B, Q), np.float32)
    for c in range(N_CORES):
        blk = np.empty((BC, Q), np.float32)
        blk[:, qr] = res.results[c]["outT"].T
        out[BC * c : BC * (c + 1), :] = blk
    return out


# revision 36
# speedup vs baseline: 1466.1888x; 1.0074x over previous
"""Trainium2 Bass kernel for nn_DivEncLayer (grouped tiny-MLP + ELU + LayerNorm + proj).

Math (per batch row b, slice q of Q=128, V=8, H=32):
    h   = elu(x[b,q,:] @ W1[q] + b1[q]);  hn = LN(h)*gamma[q]+beta[q]
    out[b,q] = hn @ W2[q] + b2[q]

Folded form (LN algebra -> 3 segmented reductions done by PE matmuls):
    g2c = gamma*W2 - mean(gamma*W2); c2 = sum(beta*W2)+b2
    y' = SIG*(elu(u)+1)  [SIG = 16^(-16/15); the +1 shift and SIG scale are
        free: sum_h g2c = 0, variance is shift-invariant, scales fold into
        the finishing constants]
    s' = sum_h y', w' = sum_h g2c*y', t2 = sum_h (y-1)^2  [centered square ->
        no cancellation blowup, fp16 stats inputs are safe]
    D = t2 - ((s'-H*SIG)/SIG)^2/H + H*eps;  out = c2 + (w'/SIG)*sqrt(H/D)

Device layout: features on partitions, batch on free dim. x is host-repacked
into 11 slices of 12 q (last 8 q) with a ones-row at slice row 96, so ONE
K=97 matmul per i-tile computes SIG*(u + b1 + 16) with no separate bias
matmul (W1 and the bias row are pre-scaled by SIG). i-tile i covers q=4i+j.

Per 512-batch supertile (8 per core, batch-parallel across 8 cores):
  - mm1: 32 [K=97, M=128, N=512] f32r matmuls -> PSUM pair tiles [128,1024]
  - ELU: ONE 8-stage custom DVE op per pair (no ACT exp): with m = relu(Src0),
        y' = max(Src0 - 15*SIG, sq(min(sq(sq(sq(m))), (16*SIG)^8)))
    i.e. a power-16 approximation (1+u/16)^16 ~ e^u clamped at the half-power
    level, reads PSUM f32, writes fp16
  - squares: ACT Square(y'/SIG - 1) -> fp16 per pair (free affine)
  - stats per i-tile: TWO fp16 matmuls: combined S+W (s rows 0..63, w rows
    64..127, i<16 -> bank SW1, else SW2) and T; bank rows
    r = 64*(g//4) + 16*t + 4*(g%4) + j for q = 16g+4t+j
  - finishing per supertile (software-pipelined into the next supertile via
    FIN_DELAY): ACT copies of SW1/SW2, 4 partition-realigning SBUF DMAs,
    custom VAR_PREP2, ACT Ln, ACT Exp (exp(-0.5 ln D + 0.5 ln H - ln SIG) =
    sqrt(H/D)/SIG), custom MUL_ADD.  All ACT functions live in
    natural_log_exp_and_others -> exactly one act-table load (enforced by
    pinning the table chooser during compile).
"""

import os
import sys

for _p in ("/opt/trn_rl_repo",):
    if _p not in sys.path:
        sys.path.insert(0, _p)

import numpy as np

B, Q, V, H = 32768, 128, 8, 32
N_CORES = 8
BC = B // N_CORES          # 4096 batch rows per core
SB = 512                   # supertile batch columns
NST = BC // SB             # 8 supertiles per core
LN_EPS = 1e-5
EA = 16.0                  # ELU poly shift/power
SIG = float(16.0 ** (-16.0 / 15.0))   # global y-scale folded into W1 (power-16 poly)
C0_ELU = float((16.0 * SIG) ** 8)     # half-power clamp level
C1_ELU = float(15.0 * SIG)            # d-branch offset

NSLICE = 11                # x slices: 10 x 12q + 1 x 8q
SROWS = 97                 # rows per slice: 12*8 x-rows + ones row at 96
XROWS = NSLICE * SROWS     # 1067

_CACHE = {}
_OPS_REGISTERED = False
_last_in_maps = None


def _q_of_r():
    # fin lane r = 64*h + 16*t + 4*gl + j with g = gl + 4*h;  q = 16*g + 4*t + j
    r = np.arange(128)
    h, t, gl, j = r // 64, (r % 64) // 16, (r % 16) // 4, r % 4
    return (16 * (gl + 4 * h) + 4 * t + j).astype(np.int64)


def _slice_of_i(i):
    """(slice index, tile-within-slice) for i-tile i (covers q = 4i..4i+3)."""
    if i < 30:
        return i // 3, i % 3
    return 10, i - 30


def _act_single_table(tables):
    """Zero out every act set except natural_log_exp_and_others (preserving
    indices) so the table-load fixpoint resolves all our ACT functions
    (Square/Copy/Ln/Exp) to one set -> a single load at kernel start."""
    keep = "natural_log_exp_and_others"
    return {name: (fns if name == keep else set()) for name, fns in tables.items()}


def _register_custom_ops():
    global _OPS_REGISTERED
    import concourse.dve_ops as dve_ops
    from concourse.dve_ops import DveOp
    from concourse.dve_spec import C0, C1, C2, One, Spec, Src0, Src1, lower, minn, maxx, relu, sq
    from concourse.dve_uop import DveOpSpec

    if _OPS_REGISTERED:
        return {op.name: op for op in dve_ops.OPS}

    def _pin(name, spec, ref):
        spec = Spec(body=spec, reference=ref)
        shas = {}
        for ver in ("v3", "v4"):
            row = dve_ops._CUSTOM_DVE_ROW_BASE + len(dve_ops.OPS)
            tmp = DveOpSpec(name=name, opcode=row, uops=lower(spec, ver=ver),
                            rd1_en=True)
            shas[ver] = tmp.sha(ver)
        op = DveOp(name, spec, subdim=False, uops_sha=shas)
        dve_ops.OPS.append(op)
        dve_ops.CUSTOM_DVE_SPECS[name] = spec
        dve_ops._SUB_OPCODE_FOR_NAME[name] = dve_ops._CUSTOM_DVE_ROW_BASE + len(dve_ops.OPS) - 1
        return op

    # y' = SIG*(elu(u)+1) approx from Src0 = SIG*(u + 16)   [SIG = 16^(-16/15)]
    #   m = relu(Src0); y' = max(Src0 - C1, sq(min(sq(sq(sq(m))), C0)))
    #   C0 = (16*SIG)^8 (half-power clamp), C1 = 15*SIG
    def _poly_ref(in0, in1, s0, s1, imm2):
        x = in0.astype(np.float32)
        m = np.maximum(x, 0.0)
        m8 = ((m * m) ** 2) ** 2
        pc = np.minimum(m8, s0)
        return np.maximum(x - s1, pc * pc)

    _pin(
        "POLY_ELU1_ANT",
        maxx(Src0 - C1, sq(minn(sq(sq(sq(relu(Src0)))), C0))),
        _poly_ref,
    )
    # D = (t - sq(s - C2)*C0) + C1   (VAR_PREP on centered stats)
    _pin(
        "VAR_PREP2_ANT",
        (Src0 - sq(Src1 - C2) * C0) + C1,
        lambda in0, in1, s0, s1, imm2: (
            in0.astype(np.float32) - (in1.astype(np.float32) - imm2) ** 2 * s0
        ) + s1,
    )
    # out = rstd * w + c2
    _pin(
        "MUL_ADD_ANT",
        Src0 * Src1 + C0,
        lambda in0, in1, s0, s1, imm2: in0.astype(np.float32) * in1 + s0,
    )
    _OPS_REGISTERED = True
    return {op.name: op for op in dve_ops.OPS}


def _build_program(tile_dt_name: str, ablate: str = "", reps: int = 1):
    ab = set(ablate.split(",")) if ablate else set()
    import concourse.bacc as bacc
    import concourse.tile as tile
    from concourse import mybir

    ops = _register_custom_ops()

    f32 = mybir.dt.float32
    f32r = mybir.dt.float32r
    bf16 = mybir.dt.bfloat16
    AF = mybir.ActivationFunctionType

    nc = bacc.Bacc(
        "TRN2",
        target_bir_lowering=False,
        debug=False,
        enable_asserts=False,
        num_devices=N_CORES,
    )

    f16 = mybir.dt.float16

    xT = nc.dram_tensor("xT", [XROWS, BC], f32r, kind="ExternalInput").ap()
    w1p = nc.dram_tensor("w1p", [SROWS, 32 * 128], f32r, kind="ExternalInput").ap()
    swc = nc.dram_tensor("swc", [128, 32 * 128], f16, kind="ExternalInput").ap()
    tsc = nc.dram_tensor("tsc", [128, 32 * 128], f16, kind="ExternalInput").ap()
    c2p = nc.dram_tensor("c2p", [128, 1], f32, kind="ExternalInput").ap()
    outT = nc.dram_tensor("outT", [128, BC], f32, kind="ExternalOutput").ap()

    with tile.TileContext(nc) as tc:
        with (
            tc.tile_pool(name="wts", bufs=1) as wts,
            tc.tile_pool(name="xt", bufs=14) as xtp,
            tc.tile_pool(name="het", bufs=4) as hetp,
            tc.tile_pool(name="sqp", bufs=4) as sqpp,
            tc.tile_pool(name="fin", bufs=2) as fin,
            tc.tile_pool(name="hep", bufs=2, space="PSUM") as hep,
            tc.tile_pool(name="stp", bufs=1, space="PSUM") as stp,
            tc.tile_pool(name="stpt", bufs=2, space="PSUM") as stpt,
        ):
            w1s = wts.tile([SROWS, 32 * 128], f32r)
            nc.sync.dma_start(out=w1s, in_=w1p)
            sws = wts.tile([128, 32 * 128], f16)
            nc.sync.dma_start(out=sws, in_=swc)
            tss = wts.tile([128, 32 * 128], f16)
            nc.sync.dma_start(out=tss, in_=tsc)
            c2s = wts.tile([128, 1], f32)
            nc.sync.dma_start(out=c2s, in_=c2p)
            zero_c = wts.tile([128, 1], f32)
            nc.vector.memset(zero_c, 0.0)
            negone_c = wts.tile([128, 1], f32)
            nc.vector.memset(negone_c, -1.0)
            lnh_c = wts.tile([128, 1], f32)
            nc.vector.memset(lnh_c, float(0.5 * np.log(H) - np.log(SIG)))

            def emit_fin(bankSW1, bankSW2, bankT, st):
                # gather s rows (low half of each 32-window) and w rows (high
                # half) from SW1 (g<4) / SW2 (g>=4) into lane-aligned SBUF
                # tiles via partition-shifting strip DMAs
                cS1 = fin.tile([128, SB], f32, tag="cS1")
                nc.scalar.activation(cS1, bankSW1, AF.Copy, bias=0.0)
                cS2 = fin.tile([128, SB], f32, tag="cS2")
                nc.scalar.activation(cS2, bankSW2, AF.Copy, bias=0.0)
                zt = fin.tile([128, SB], f32, tag="zt")
                wt = fin.tile([128, SB], f32, tag="wt")
                nc.sync.dma_start(out=zt[0:64, :], in_=cS1[0:64, :])
                nc.sync.dma_start(out=zt[64:128, :], in_=cS2[0:64, :])
                nc.sync.dma_start(out=wt[0:64, :], in_=cS1[64:128, :])
                nc.sync.dma_start(out=wt[64:128, :], in_=cS2[64:128, :])
                # D = t2 - ((s-H*SIG)/SIG)^2/H + H*eps; out = c2 + (w/SIG)*sqrt(H/D)
                Dt = fin.tile([128, SB], f32, tag="Dt")
                nc.vector._custom_dve(
                    ops["VAR_PREP2_ANT"], out=Dt, in0=bankT, in1=zt,
                    s0=float(1.0 / (H * SIG * SIG)), s1=float(H * LN_EPS),
                    imm2=float(H * SIG),
                )
                Lt = fin.tile([128, SB], f32, tag="Lt")
                nc.scalar.activation(Lt, Dt, AF.Ln, bias=zero_c[:, 0:1])
                rstd = fin.tile([128, SB], f32, tag="rstd")
                nc.scalar.activation(rstd, Lt, AF.Exp, bias=lnh_c[:, 0:1], scale=-0.5)
                of = fin.tile([128, SB], f32, tag="of")
                nc.vector._custom_dve(
                    ops["MUL_ADD_ANT"], out=of, in0=rstd, in1=wt,
                    s0=c2s[:, 0:1], s1=0.0,
                )
                nc.sync.dma_start(out=outT[:, SB * st : SB * st + SB], in_=of)

            FIN_DELAY = 4   # i-tiles of the next supertile emitted before fin
            for _rep in range(reps):
              pending = None
              xts2 = None
              for st in range(NST):
                if st % 2 == 0:
                    xts2 = []
                    for s in range(NSLICE):
                        xt_t = xtp.tile([SROWS, 2 * SB], f32r, tag="xt")
                        nc.sync.dma_start(
                            out=xt_t,
                            in_=xT[
                                SROWS * s : SROWS * s + SROWS,
                                SB * st : SB * st + 2 * SB,
                            ],
                        )
                        xts2.append(xt_t)
                xts = [xt[:, SB * (st % 2) : SB * (st % 2) + SB] for xt in xts2]

                bankSW1 = stp.tile([128, SB], f32, tag="bankSW1")
                bankSW2 = stp.tile([128, SB], f32, tag="bankSW2")
                bankT = stpt.tile([128, SB], f32, tag="bankT")

                def emit_stats(i, hsl, qsl):
                    bsw = bankSW1 if i < 16 else bankSW2
                    nc.tensor.matmul(
                        bsw, lhsT=sws[:, 128 * i : 128 * i + 128], rhs=hsl,
                        start=(i % 16 == 0), stop=(i % 16 == 15),
                    )
                    nc.tensor.matmul(
                        bankT, lhsT=tss[:, 128 * i : 128 * i + 128], rhs=qsl,
                        start=(i == 0), stop=(i == 31),
                    )

                pend_stats = []
                for pr in range(16):
                    he = hep.tile([128, 2 * SB], f32, tag="he")
                    for z in range(2):
                        i = 2 * pr + z
                        s, _tl = _slice_of_i(i)
                        nc.tensor.matmul(
                            he[:, SB * z : SB * z + SB],
                            lhsT=w1s[:, 128 * i : 128 * i + 128],
                            rhs=xts[s],
                            start=True,
                            stop=True,
                        )
                    het1 = hetp.tile([128, 2 * SB], f16, tag="het1")
                    nc.vector._custom_dve(
                        ops["POLY_ELU1_ANT"], out=het1, in0=he,
                        s0=C0_ELU, s1=C1_ELU,
                    )
                    # (y'/SIG - 1)^2 = (y-1)^2 in fp16 via ACT free affine
                    sq1 = sqpp.tile([128, 2 * SB], f16, tag="sq1")
                    nc.scalar.activation(
                        sq1, het1, AF.Square, bias=negone_c[:, 0:1],
                        scale=float(1.0 / SIG),
                    )

                    for z in range(2):
                        i = 2 * pr + z
                        hsl = het1[:, SB * z : SB * z + SB]
                        qsl = sq1[:, SB * z : SB * z + SB]
                        if pending is not None and pr < FIN_DELAY:
                            pend_stats.append(
                                lambda i=i, h=hsl, q=qsl: emit_stats(i, h, q)
                            )
                        else:
                            if pending is not None:
                                emit_fin(*pending)
                                pending = None
                                for f in pend_stats:
                                    f()
                                pend_stats = []
                            emit_stats(i, hsl, qsl)

                pending = (bankSW1, bankSW2, bankT, st)
              emit_fin(*pending)

    # Compile with the act-table chooser pinned to one set so walrus emits a
    # single InstLoadActFuncSet instead of ping-ponging between sets.
    import concourse.hw_specs as hw_specs

    orig = hw_specs.get_activation_tables
    patched = lambda arch: _act_single_table(orig(arch))
    hw_specs_get = hw_specs.get_activation_tables
    try:
        hw_specs.get_activation_tables = patched
        bacc.get_activation_tables = patched
        nc.compile()
    finally:
        hw_specs.get_activation_tables = hw_specs_get
        bacc.get_activation_tables = hw_specs_get
    return nc


def _host_pack(W1, b1, gamma, beta, W2, b2):
    f16dt = np.float16
    g2 = (gamma * W2[:, :, 0]).astype(np.float64)
    g2c = (g2 - g2.sum(-1, keepdims=True) / H).astype(np.float32)
    c2 = ((beta * W2[:, :, 0]).sum(-1) + b2[:, 0]).astype(np.float32)

    sig = np.float32(SIG)
    w1p = np.zeros((SROWS, 32 * 128), np.float32)
    swc = np.zeros((128, 32 * 128), np.float32)
    tsc = np.zeros((128, 32 * 128), np.float32)
    for i in range(32):
        s, tl = _slice_of_i(i)
        g, t = i // 4, i % 4
        gl = g % 4
        for j in range(4):
            q = 4 * i + j
            ql = 4 * tl + j                      # q-local index within slice
            w1p[8 * ql : 8 * ql + 8, 128 * i + 32 * j : 128 * i + 32 * j + 32] = (
                W1[q] * sig
            )
            w1p[96, 128 * i + 32 * j : 128 * i + 32 * j + 32] = sig * (b1[q] + EA)
            rs = 16 * t + 4 * gl + j             # s row (0..63) in SW bank
            r = 64 * (g // 4) + rs               # fin lane for q
            swc[32 * j : 32 * j + 32, 128 * i + rs] = 1.0          # s selector
            swc[32 * j : 32 * j + 32, 128 * i + 64 + rs] = g2c[q]  # w selector
            tsc[32 * j : 32 * j + 32, 128 * i + r] = 1.0           # t selector
    c2p = c2[_q_of_r()].reshape(128, 1).astype(np.float32)
    return (w1p, swc.astype(f16dt), tsc.astype(f16dt), c2p)


def _pack_x(xc):
    """[BC, 1024] core slice -> [XROWS, BC] with ones rows at slice row 96."""
    xp = np.zeros((XROWS, xc.shape[0]), np.float32)
    xcT = xc.T  # [1024, BC]; original row = 8*q + v
    for s in range(NSLICE):
        nq = 12 if s < 10 else 8
        q0 = 12 * s
        src = xcT[8 * q0 : 8 * (q0 + nq), :]
        xp[SROWS * s : SROWS * s + 8 * nq, :] = src
        xp[SROWS * s + 96, :] = 1.0
    return xp


def kernel(x, W1, b1, gamma, beta, W2, b2):
    from concourse import bass_utils

    tile_dt_name = os.environ.get("KERNEL_TILE_DT", "bfloat16")
    key = tile_dt_name
    if key not in _CACHE:
        _CACHE[key] = _build_program(tile_dt_name)
    nc = _CACHE[key]

    x = np.asarray(x, np.float32)
    w1p, swc, tsc, c2p = _host_pack(
        np.asarray(W1, np.float32),
        np.asarray(b1, np.float32),
        np.asarray(gamma, np.float32),
        np.asarray(beta, np.float32),
        np.asarray(W2, np.float32),
        np.asarray(b2, np.float32),
    )

    in_maps = []
    for c in range(N_CORES):
        xc = x[BC * c : BC * (c + 1), :]          # [4096, 1024]
        in_maps.append(
            {
                "xT": _pack_x(xc),
                "w1p": w1p,
                "swc": swc,
                "tsc": tsc,
                "c2p": c2p,
            }
        )

    global _last_in_maps
    _last_in_maps = in_maps

    res = bass_utils.run_bass_kernel_spmd(
        nc, in_maps, core_ids=list(range(N_CORES))
    )

    qr = _q_of_r()
    out = np.empty((B, Q), np.float32)
    for c in range(N_CORES):
        blk = np.empty((BC, Q), np.float32)
        blk[:, qr] = res.results[c]["outT"].T
        out[BC * c : BC * (c + 1), :] = blk
    return out
